# revision 1
# baseline (speedup 1.0000x reference)
"""LoRA Linear (x @ W.T + b + scaling * (x @ A.T) @ B.T) on 8 TRN2 NeuronCores.

Strategy:
  - Fold the LoRA adapter into the dense weight on host:
        Wf = W + (alpha/rank) * (lora_B @ lora_A)        (exact algebra)
    so the device kernel is a single dense matmul + bias.
  - Data-parallel: shard the 8192 tokens into 8 x 1024 rows, one shard per core.
    W is replicated (sharding_hint).
  - Per core: out[m, o] = sum_k xT[k, m] * WT[k, o] + b[o]
    PE matmul tiles: lhsT = xT[k128, m128] (stationary), rhs = WT[k128, o512]
    (moving), accumulate 32 k-tiles into a [128, 512] PSUM bank, DVE adds the
    (pre-broadcast) bias while copying PSUM -> SBUF, DMA to out.
  - bf16 matmul operands (PE runs bf16 at full rate, fp32 at 1/4 rate),
    fp32 PSUM accumulation.
"""

import numpy as np
import ml_dtypes

import concourse.bass as bass
from concourse import bacc
import concourse.mybir as mybir
import concourse.tile as tile
from concourse.bass_utils import run_bass_kernel_spmd

N_CORES = 8
IN_F = 4096
OUT_F = 4096
RANK = 16
ALPHA = 32.0
B_SZ = 4
S_SZ = 2048
TOK = B_SZ * S_SZ          # 8192
M_PER_CORE = TOK // N_CORES  # 1024

P = 128                    # partitions
KT = IN_F // P             # 32 k-tiles
O_BLK = 512                # o-block width (matmul moving free dim)
N_OBLK = OUT_F // O_BLK    # 8
MT = M_PER_CORE // P       # 8 m-tiles

MM_DT = mybir.dt.bfloat16
NP_MM_DT = ml_dtypes.bfloat16

LAST_RESULTS = None        # test.py reads exec_time_ns from here


KTB = KT + 1               # extra k-tile carries the bias row


def _build_nc(trace_scopes=False):
    nc = bacc.Bacc(None, target_bir_lowering=False)

    xt_d = nc.dram_tensor("xt", [KTB, P, M_PER_CORE], MM_DT, kind="ExternalInput")
    wt_d = nc.dram_tensor("wt", [N_OBLK, KTB, P, O_BLK], MM_DT, kind="ExternalInput")
    out_d = nc.dram_tensor("out", [M_PER_CORE, OUT_F], mybir.dt.float32,
                           kind="ExternalOutput")

    with tile.TileContext(nc) as tc:
        with (
            tc.tile_pool(name="xt", bufs=1) as xt_pool,
            tc.tile_pool(name="wt", bufs=2) as wt_pool,
            tc.tile_pool(name="outs", bufs=8) as out_pool,
            tc.tile_pool(name="psum", bufs=8, space="PSUM") as psum_pool,
        ):
            # Per-k-tile loads (instead of one monolithic DMA) so the first
            # matmuls only wait on their own k-slice: ~18 us faster startup.
            xts = []
            for k in range(KTB):
                t = xt_pool.tile([P, M_PER_CORE], MM_DT, tag=f"xt{k}")
                nc.sync.dma_start(t[:], xt_d[k])
                xts.append(t)

            for ob in range(N_OBLK):
                wts = []
                for k in range(KTB):
                    t = wt_pool.tile([P, O_BLK], MM_DT, tag=f"wt{k}")
                    nc.sync.dma_start(t[:], wt_d[ob, k])
                    wts.append(t)

                for mt in range(MT):
                    psum = psum_pool.tile([P, O_BLK], mybir.dt.float32)
                    for k in range(KTB):
                        nc.tensor.matmul(
                            psum[:],
                            xts[k][:, mt * P:(mt + 1) * P],
                            wts[k][:],
                            start=(k == 0),
                            stop=(k == KTB - 1),
                        )
                    out_sb = out_pool.tile([P, O_BLK], mybir.dt.float32)
                    nc.any.tensor_copy(out=out_sb[:], in_=psum[:])
                    nc.sync.dma_start(
                        out_d[mt * P:(mt + 1) * P, ob * O_BLK:(ob + 1) * O_BLK],
                        out_sb[:],
                    )
    nc.compile()
    return nc


_NC_CACHE = None


def kernel(x, W, b, lora_A, lora_B, _trace=False):
    global LAST_RESULTS, _NC_CACHE

    # ---- host prep ----
    scaling = ALPHA / RANK
    Wf = (W.astype(np.float64)
          + scaling * (lora_B.astype(np.float64) @ lora_A.astype(np.float64)))
    # WT[k, o] = Wf[o, k]; pre-tiled into o-blocks: [N_OBLK, KT, P, O_BLK],
    # plus one extra k-tile whose partition-0 row carries the bias.
    WT = np.ascontiguousarray(Wf.T).astype(NP_MM_DT)          # [IN_F, OUT_F]
    wt_in = np.zeros((N_OBLK, KTB, P, O_BLK), dtype=NP_MM_DT)
    wt_in[:, :KT] = WT.reshape(KT, P, N_OBLK, O_BLK).transpose(2, 0, 1, 3)
    b_blk = b.astype(np.float32).reshape(N_OBLK, O_BLK)
    wt_in[:, KT, 0, :] = b_blk.astype(NP_MM_DT)

    x_flat = np.ascontiguousarray(x.reshape(TOK, IN_F))

    in_maps = []
    for c in range(N_CORES):
        xc = x_flat[c * M_PER_CORE:(c + 1) * M_PER_CORE]       # [1024, 4096]
        xt = np.ascontiguousarray(xc.T).astype(NP_MM_DT)       # [4096, 1024]
        xt_in = np.zeros((KTB, P, M_PER_CORE), dtype=NP_MM_DT)
        xt_in[:KT] = xt.reshape(KT, P, M_PER_CORE)
        xt_in[KT, 0, :] = 1.0                                  # bias one-hot row
        in_maps.append({
            "xt": xt_in,
            "wt": wt_in,
        })

    if _NC_CACHE is None:
        _NC_CACHE = _build_nc()
    nc = _NC_CACHE

    res = run_bass_kernel_spmd(nc, in_maps, core_ids=list(range(N_CORES)),
                               trace=_trace)
    LAST_RESULTS = res

    out = np.concatenate([r["out"] for r in res.results], axis=0)
    return out.reshape(B_SZ, S_SZ, OUT_F).astype(np.float32)



# revision 5
# speedup vs baseline: 3.2666x; 3.2666x over previous
"""LoRA Linear (x @ W.T + b + (alpha/rank) * (x @ A.T) @ B.T) on 8 TRN2 cores.

Strategy (v2, fp8 DoubleRow):
  - Data-parallel over tokens: 8192 tokens -> 8 shards of 1024 rows.
  - Base matmul x @ W.T runs in fp8e4 with perf_mode=DoubleRow: each matmul
    contracts K=256 (128 partitions x 2 lanes) into a [128, 512] PSUM tile at
    0.5 cycles/row -- 4x fewer PE cycles than the bf16 baseline.
  - The LoRA adapter is NOT folded into W (fp8 quantization of the folded
    weight is too coarse: the rank-16 term dominates the output). Instead the
    adapter runs on-device in high precision:
      stage 1: xa = x @ A.T  via tiny matmuls (out free dim = 16): stationary
               x_hi/x_lo fp8 slices, moving A in bf16.
      transpose: xa [128m, 16r] -> xaT [16r, 128m] on the PE (identity mm).
      stage 2: per out-tile [128, 512]: one bf16 matmul with K=17
               (16 rank lanes + 1 ones lane carrying the bias), accumulated
               into the same PSUM group as the base matmuls (ob >= 2) or into
               a separate PSUM tile + DVE add (ob 0-1, whose base groups
               finish before xa is ready).
  - x ships as an fp8 hi+lo pair (x ~ x_hi + x_lo, both e4m3): x_hi feeds the
    base matmul, hi+lo feed stage 1. Same DMA bytes as bf16 x.
  - W ships as fp8(8*W) to avoid e4m3 subnormals (W ~ N(0, 1/64)); the
    adapter/bias are pre-scaled by 8 as well and the host multiplies the
    bf16 output by 0.125 (exact power-of-2) after gathering.
"""

import numpy as np

import concourse.bass as bass
from concourse import bacc
import concourse.mybir as mybir
import concourse.tile as tile
from concourse.bass_utils import run_bass_kernel_spmd

N_CORES = 8
IN_F = 4096
OUT_F = 4096
RANK = 16
ALPHA = 32.0
B_SZ = 4
S_SZ = 2048
TOK = B_SZ * S_SZ            # 8192
M_PER_CORE = TOK // N_CORES  # 1024

P = 128
KT2 = IN_F // 256            # 16 DoubleRow k-tiles (256 contraction each)
O_BLK = 512
N_OBLK = OUT_F // O_BLK      # 8
MT = M_PER_CORE // P         # 8

FP8 = mybir.dt.float8e4
NP_FP8 = mybir.dt.np(FP8)
BF16 = mybir.dt.bfloat16
NP_BF16 = mybir.dt.np(BF16)
F32 = mybir.dt.float32

W_SCALE = 8.0                # device computes 8x the result; host undoes it

LAST_RESULTS = None          # test.py reads exec_time_ns from here

NCHUNK = 4                   # DMA chunks for xh/xl/wt tensors (along kt2)
TPC = KT2 // NCHUNK          # kt2 tiles per chunk


def _build_nc():
    nc = bacc.Bacc(None, target_bir_lowering=False)

    xh_d = nc.dram_tensor("xh", [P, KT2 * 2 * M_PER_CORE], FP8, kind="ExternalInput")
    xl_d = nc.dram_tensor("xl", [P, KT2 * 2 * M_PER_CORE], FP8, kind="ExternalInput")
    wt_d = nc.dram_tensor("wt", [N_OBLK, P, KT2 * 2 * O_BLK], FP8, kind="ExternalInput")
    at_d = nc.dram_tensor("at", [P, 32 * RANK], BF16, kind="ExternalInput")
    badp_d = nc.dram_tensor("badp", [RANK + 1, OUT_F], BF16, kind="ExternalInput")
    ident_d = nc.dram_tensor("ident", [P, P], BF16, kind="ExternalInput")
    ones_d = nc.dram_tensor("ones", [1, M_PER_CORE], BF16, kind="ExternalInput")
    out_d = nc.dram_tensor("out", [M_PER_CORE, OUT_F], BF16, kind="ExternalOutput")

    with tile.TileContext(nc) as tc:
        with (
            tc.tile_pool(name="sb", bufs=1) as sb,
            tc.tile_pool(name="wtp", bufs=2) as wtp,
            tc.tile_pool(name="outp", bufs=20) as outp,
            tc.tile_pool(name="xap", bufs=2) as xap,
            tc.tile_pool(name="psb", bufs=4, space="PSUM") as psb,
            tc.tile_pool(name="psa", bufs=1, space="PSUM") as psa,
            tc.tile_pool(name="psx", bufs=2, space="PSUM") as psx,
            tc.tile_pool(name="pst", bufs=1, space="PSUM") as pst,
        ):
            # ---- persistent SBUF tensors ----
            xh_sb = sb.tile([P, KT2, 2, M_PER_CORE], FP8, tag="xh")
            xl_sb = sb.tile([P, KT2, 2, M_PER_CORE], FP8, tag="xl")
            at_sb = sb.tile([P, 32 * RANK], BF16, tag="at")
            ident_sb = sb.tile([P, P], BF16, tag="ident")
            badp_sb = sb.tile([RANK + 1, OUT_F], BF16, tag="badp")
            xaT_sb = sb.tile([RANK + 1, M_PER_CORE], BF16, tag="xaT")

            xcsz = TPC * 2 * M_PER_CORE   # elements per xh/xl DMA chunk
            wcsz = TPC * 2 * O_BLK        # elements per wt DMA chunk

            def load_x_chunk(dst, src, c):
                nc.sync.dma_start(
                    dst[:, c * TPC:(c + 1) * TPC].rearrange("p t i m -> p (t i m)"),
                    src[:, c * xcsz:(c + 1) * xcsz],
                )

            wt_tiles = {}

            def load_wt(ob, interleave_with=None):
                t = wtp.tile([P, KT2, 2, O_BLK], FP8, tag="wt")
                for c in range(NCHUNK):
                    nc.sync.dma_start(
                        t[:, c * TPC:(c + 1) * TPC].rearrange("p t i m -> p (t i m)"),
                        wt_d[ob, :, c * wcsz:(c + 1) * wcsz],
                    )
                    if interleave_with is not None:
                        dst, src = interleave_with
                        load_x_chunk(dst, src, c)
                wt_tiles[ob] = t

            # ---- DMA priority order (sync queue is FIFO) ----
            load_wt(0, interleave_with=(xh_sb, xh_d))   # wt0 & xh interleaved
            nc.sync.dma_start(at_sb[:], at_d[:])
            nc.sync.dma_start(ident_sb[:], ident_d[:])
            nc.sync.dma_start(badp_sb[:], badp_d[:])
            nc.sync.dma_start(xaT_sb[RANK:RANK + 1, :], ones_d[:])
            load_wt(1)
            for c in range(NCHUNK):
                load_x_chunk(xl_sb, xl_d, c)
            load_wt(2)
            # wt3..wt7 are emitted inside the ob loop (after xl in program
            # order, so the waiting wt DMAs never block the xl load).

            # ---- compute ----
            adp_backlog = []   # (mt, ob, out_t) awaiting adapter matmul + add

            def base_mm(pb, mt, wt_t, t, start, stop):
                nc.tensor.matmul(
                    pb[:],
                    xh_sb[:, t, :, mt * P:(mt + 1) * P],
                    wt_t[:, t, :, :],
                    start=start, stop=stop,
                    perf_mode=mybir.MatmulPerfMode.DoubleRow,
                )

            def adapter_mm(ps, mt, ob, start, stop):
                nc.tensor.matmul(
                    ps[:],
                    xaT_sb[:, mt * P:(mt + 1) * P],
                    badp_sb[:, ob * O_BLK:(ob + 1) * O_BLK],
                    start=start, stop=stop,
                )

            def stage1():
                xa_list = []
                for mt in range(MT):
                    px = psx.tile([P, RANK], F32, tag="px")
                    n = 0
                    for kt in range(32):
                        t, i = kt // 2, kt % 2
                        for xsb in (xh_sb, xl_sb):
                            nc.tensor.matmul(
                                px[:],
                                xsb[:, t, i, mt * P:(mt + 1) * P],
                                at_sb[:, kt * RANK:(kt + 1) * RANK],
                                start=(n == 0), stop=(n == 63),
                            )
                            n += 1
                    xa_t = xap.tile([P, RANK], BF16, tag="xa")
                    nc.vector.tensor_copy(out=xa_t[:], in_=px[:])
                    xa_list.append(xa_t)
                return xa_list

            def transpose_one(mt, xa_t):
                pt = pst.tile([RANK, P], BF16, tag="pt")
                nc.tensor.transpose(pt[:], xa_t[:], ident_sb[:])
                nc.vector.tensor_copy(
                    out=xaT_sb[0:RANK, mt * P:(mt + 1) * P], in_=pt[:])

            def store(out_t, mt, ob):
                nc.sync.dma_start(
                    out_d[mt * P:(mt + 1) * P, ob * O_BLK:(ob + 1) * O_BLK],
                    out_t[:])

            def drain_backlog(k=1):
                for _ in range(k):
                    if not adp_backlog:
                        return
                    mt, ob, out_t = adp_backlog.pop(0)
                    pa = psa.tile([P, O_BLK], F32, tag="pa")
                    adapter_mm(pa, mt, ob, True, True)
                    nc.vector.tensor_tensor(
                        out=out_t[:], in0=out_t[:], in1=pa[:],
                        op=mybir.AluOpType.add)
                    store(out_t, mt, ob)

            # ob 0: k-outer emission over mt quads so matmuls start as soon
            # as the first xh/wt0 chunks land (groups decoupled from adapter).
            wt_t = wt_tiles[0]
            for mtq in range(0, MT, 4):
                pbs = [psb.tile([P, O_BLK], F32, tag="pb", name=f"pb{mtq}_{j}")
                       for j in range(4)]
                for t in range(KT2):
                    for j, mt in enumerate(range(mtq, mtq + 4)):
                        base_mm(pbs[j], mt, wt_t, t,
                                start=(t == 0), stop=(t == KT2 - 1))
                for j, mt in enumerate(range(mtq, mtq + 4)):
                    out_t = outp.tile([P, O_BLK], BF16, tag="out")
                    nc.scalar.copy(out=out_t[:], in_=pbs[j][:])
                    adp_backlog.append((mt, 0, out_t))

            # ob 1: normal order, still decoupled from the adapter.
            wt_t = wt_tiles[1]
            for mt in range(MT):
                pb = psb.tile([P, O_BLK], F32, tag="pb")
                for t in range(KT2):
                    base_mm(pb, mt, wt_t, t, start=(t == 0), stop=(t == KT2 - 1))
                out_t = outp.tile([P, O_BLK], BF16, tag="out")
                nc.scalar.copy(out=out_t[:], in_=pb[:])
                adp_backlog.append((mt, 1, out_t))

            # stage 1 (xa accumulation) between ob1 and ob2: xl has arrived.
            xa_list = stage1()

            # obs 2..7: adapter accumulated in-group (17th matmul).
            for ob in range(2, N_OBLK):
                if ob + 1 < N_OBLK and (ob + 1) not in wt_tiles:
                    load_wt(ob + 1)
                wt_t = wt_tiles[ob]
                for mt in range(MT):
                    pb = psb.tile([P, O_BLK], F32, tag="pb")
                    for t in range(KT2):
                        base_mm(pb, mt, wt_t, t, start=(t == 0), stop=False)
                    if ob == 2:
                        transpose_one(mt, xa_list[mt])
                    adapter_mm(pb, mt, ob, False, True)
                    out_t = outp.tile([P, O_BLK], BF16, tag="out")
                    nc.scalar.copy(out=out_t[:], in_=pb[:])
                    store(out_t, mt, ob)
                    drain_backlog(1)
            drain_backlog(len(adp_backlog))

    nc.compile()
    return nc


_NC_CACHE = None


def kernel(x, W, b, lora_A, lora_B, _trace=False):
    global LAST_RESULTS, _NC_CACHE

    scaling = ALPHA / RANK
    x = np.asarray(x, dtype=np.float32)
    W = np.asarray(W, dtype=np.float32)
    b = np.asarray(b, dtype=np.float32)
    A = np.asarray(lora_A, dtype=np.float32)
    B = np.asarray(lora_B, dtype=np.float32)

    # ---- weights (shared across cores) ----
    W8T = np.ascontiguousarray((W_SCALE * W).T).astype(NP_FP8)   # [IN_F, OUT_F]
    # wt[ob, p, t, i, o] = W8T[t*256 + i*128 + p, ob*512 + o]
    wt_in = np.ascontiguousarray(
        W8T.reshape(KT2, 2, P, N_OBLK, O_BLK).transpose(3, 2, 0, 1, 4)
    ).reshape(N_OBLK, P, KT2 * 2 * O_BLK)

    # at[p, kt*16 + r] = A[r, kt*128 + p]
    at_in = np.ascontiguousarray(
        A.T.reshape(32, P, RANK).transpose(1, 0, 2)
    ).reshape(P, 32 * RANK).astype(NP_BF16)

    badp_in = np.empty((RANK + 1, OUT_F), dtype=NP_BF16)
    badp_in[:RANK] = (W_SCALE * scaling) * B.T
    badp_in[RANK] = W_SCALE * b

    ident_in = np.eye(P, dtype=NP_BF16)
    ones_in = np.ones((1, M_PER_CORE), dtype=NP_BF16)

    # ---- per-core x shards (fp8 hi + lo) ----
    x_flat = np.ascontiguousarray(x.reshape(TOK, IN_F))

    def pack_x(a):
        # a: [IN_F, M] -> [p, t, i, m] flattened
        return np.ascontiguousarray(
            a.reshape(KT2, 2, P, M_PER_CORE).transpose(2, 0, 1, 3)
        ).reshape(P, KT2 * 2 * M_PER_CORE)

    in_maps = []
    for c in range(N_CORES):
        xs = np.ascontiguousarray(
            x_flat[c * M_PER_CORE:(c + 1) * M_PER_CORE].T)   # [IN_F, M]
        x8h = xs.astype(NP_FP8)
        x8l = (xs - x8h.astype(np.float32)).astype(NP_FP8)
        in_maps.append({
            "xh": pack_x(x8h),
            "xl": pack_x(x8l),
            "wt": wt_in,
            "at": at_in,
            "badp": badp_in,
            "ident": ident_in,
            "ones": ones_in,
        })

    if _NC_CACHE is None:
        _NC_CACHE = _build_nc()
    nc = _NC_CACHE

    res = run_bass_kernel_spmd(nc, in_maps, core_ids=list(range(N_CORES)),
                               trace=_trace)
    LAST_RESULTS = res

    out = np.concatenate(
        [r["out"].astype(np.float32) for r in res.results], axis=0)
    out *= 1.0 / W_SCALE
    return out.reshape(B_SZ, S_SZ, OUT_F)


# revision 33
# speedup vs baseline: 3.5141x; 1.0758x over previous
"""LoRA Linear (x @ W.T + b + (alpha/rank) * (x @ A.T) @ B.T) on 8 TRN2 cores.

Strategy (v3, fp8 DoubleRow everywhere):
  - Data-parallel over tokens: 8192 tokens -> 8 shards of 1024 rows.
  - Base matmul x @ W.T in fp8e4 perf_mode=DoubleRow: each matmul contracts
    K=256 (128 partitions x 2 lanes) into a [128, 512] PSUM tile at 0.5
    cycles/row -- 4x fewer PE-array cycles than the bf16 baseline. 1024
    matmuls/core. (The PE sequencer costs ~88ns per matmul+ldweights pair in
    the cost model, so total matmul COUNT is kept low everywhere.)
  - The LoRA adapter is NOT folded into W (fp8 quantization of the folded
    weight is too coarse: the rank-16 adapter dominates the output). Instead:
      stage 1: xaT = A-parts @ x directly in transposed layout: stationary =
               packed fp8 [A_hi | A_lo] (32 rank lanes), moving = x_hi (and a
               second set: A_hi x x_lo), DoubleRow, out [32, 512] PSUM.
               64 matmuls, no transposes needed.
      split:   DVE scales px by 0.25 and splits into an fp8 hi+lo pair,
               packed into the stage-2 stationary Sx [65, 2, 1024]
               (2 xa-parts x {hi,lo} x 16 ranks, + a ones lane for bias).
      stage 2: per out-tile: ONE DoubleRow fp8 matmul with 65 partitions
               (129 used lanes: xa-parts x B_{hi,lo} + bias), accumulated
               into the base PSUM group (ob >= 4) or a separate PSUM tile +
               DVE add (ob 0-3, whose base groups finish before Sx is ready).
  - x ships as an fp8 hi+lo pair (x ~ x_hi + x_lo): x_hi feeds the base
    matmul, hi+lo feed stage 1. Same DMA bytes as bf16 x.
  - W ships as fp8(8*W) to avoid e4m3 subnormals (W ~ N(0, 1/64)); B ships
    as fp8 hi/lo of 64*B, bias as fp8 hi/lo of 8*b, A as fp8 hi/lo of A.
    The device output is 8x the result in bf16; the host multiplies by
    0.125 (exact power-of-2) after gathering.
"""

import numpy as np

import concourse.bass as bass
from concourse import bacc
import concourse.mybir as mybir
import concourse.tile as tile
from concourse.bass_utils import run_bass_kernel_spmd

N_CORES = 8
IN_F = 4096
OUT_F = 4096
RANK = 16
ALPHA = 32.0
B_SZ = 4
S_SZ = 2048
TOK = B_SZ * S_SZ            # 8192
M_PER_CORE = TOK // N_CORES  # 1024

P = 128
KT2 = IN_F // 256            # 16 DoubleRow k-tiles (256 contraction each)
O_BLK = 512
N_OBLK = OUT_F // O_BLK      # 8
MT = M_PER_CORE // P         # 8
SXP = 4 * RANK + 1           # 65 stage-2 stationary partitions

FP8 = mybir.dt.float8e4
NP_FP8 = mybir.dt.np(FP8)
BF16 = mybir.dt.bfloat16
NP_BF16 = mybir.dt.np(BF16)
F32 = mybir.dt.float32
DR = mybir.MatmulPerfMode.DoubleRow

W_SCALE = 8.0                # device computes 8x the result; host undoes it

LAST_RESULTS = None          # test.py reads exec_time_ns from here

NCHUNK = 4                   # DMA chunks for xl/wt tensors (along kt2)
TPC = KT2 // NCHUNK          # kt2 tiles per chunk
NCHUNK_F = 8                 # finer chunks for the feed-critical loads
TPC_F = KT2 // NCHUNK_F
TPC_F0 = 1                   # finest chunks for the very first loads (ob0)

IN_GROUP_OBS = range(4, N_OBLK)   # adapter accumulated in the base group


def _build_nc():
    nc = bacc.Bacc(None, target_bir_lowering=False)

    xh_d = nc.dram_tensor("xh", [P, KT2 * 2 * M_PER_CORE], FP8, kind="ExternalInput")
    xl_d = nc.dram_tensor("xl", [P, KT2 * 2 * M_PER_CORE], FP8, kind="ExternalInput")
    wt_d = nc.dram_tensor("wt", [N_OBLK, P, KT2 * 2 * O_BLK], FP8, kind="ExternalInput")
    at_d = nc.dram_tensor("at", [P, KT2 * 2 * 2 * RANK], FP8, kind="ExternalInput")
    badp_d = nc.dram_tensor("badp", [SXP, N_OBLK * 2 * O_BLK], FP8, kind="ExternalInput")
    ones_d = nc.dram_tensor("ones", [1, 2 * M_PER_CORE], FP8, kind="ExternalInput")
    out_d = nc.dram_tensor("out", [M_PER_CORE, OUT_F], BF16, kind="ExternalOutput")

    with tile.TileContext(nc) as tc:
        with (
            tc.tile_pool(name="sb", bufs=1) as sb,
            tc.tile_pool(name="wtp", bufs=2) as wtp,
            tc.tile_pool(name="outp", bufs=36) as outp,
            tc.tile_pool(name="psb", bufs=5, space="PSUM") as psb,
            tc.tile_pool(name="psa", bufs=1, space="PSUM") as psa,
            tc.tile_pool(name="psx", bufs=2, space="PSUM") as psx,
        ):
            # ---- persistent SBUF tensors ----
            xh_sb = sb.tile([P, KT2, 2, M_PER_CORE], FP8, tag="xh")
            xl_sb = sb.tile([P, KT2, 2, M_PER_CORE], FP8, tag="xl")
            # at[:, t, :, 0:16] = A_hi lanes, [..., 16:32] = A_lo lanes
            at_sb = sb.tile([P, KT2, 2, 2 * RANK], FP8, tag="at")
            badp_sb = sb.tile([SXP, N_OBLK, 2, O_BLK], FP8, tag="badp")
            sx_sb = sb.tile([SXP, 2, M_PER_CORE], FP8, tag="sx")
            tmp_sb = sb.tile([2 * RANK, O_BLK], F32, tag="tmp")

            xcsz = TPC * 2 * M_PER_CORE   # elements per xh/xl DMA chunk
            wcsz = TPC * 2 * O_BLK        # elements per wt DMA chunk

            def load_x_chunk(dst, src, c, tpc=TPC):
                xsz = tpc * 2 * M_PER_CORE
                nc.sync.dma_start(
                    dst[:, c * tpc:(c + 1) * tpc].rearrange("p t i m -> p (t i m)"),
                    src[:, c * xsz:(c + 1) * xsz],
                )

            wt_tiles = {}

            def load_wt(ob, interleave_with=None, tpc=TPC,
                        after_first_chunk=None):
                t = wtp.tile([P, KT2, 2, O_BLK], FP8, tag="wt")
                wsz = tpc * 2 * O_BLK
                for c in range(KT2 // tpc):
                    if interleave_with is not None:
                        dst, src = interleave_with
                        load_x_chunk(dst, src, c, tpc=tpc)
                    nc.sync.dma_start(
                        t[:, c * tpc:(c + 1) * tpc].rearrange("p t i m -> p (t i m)"),
                        wt_d[ob, :, c * wsz:(c + 1) * wsz],
                    )
                    if c == 0 and after_first_chunk is not None:
                        after_first_chunk()
                wt_tiles[ob] = t

            # ---- DMA priority order (sync queue is FIFO) ----
            # first xh/wt0 chunk pair ahead of the (small) at load so the
            # first matmuls start as early as possible
            load_wt(0, interleave_with=(xh_sb, xh_d), tpc=TPC_F,
                    after_first_chunk=lambda: nc.sync.dma_start(
                        at_sb[:].rearrange("p t i r -> p (t i r)"), at_d[:]))
            load_wt(1, tpc=TPC_F)
            load_wt(2)
            for c in range(NCHUNK):
                load_x_chunk(xl_sb, xl_d, c)
            nc.sync.dma_start(
                badp_sb[:].rearrange("p ob i o -> p (ob i o)"), badp_d[:])
            nc.sync.dma_start(
                sx_sb[SXP - 1:SXP, :, :].rearrange("p i m -> p (i m)"), ones_d[:])
            load_wt(3)
            # wt4..wt7 are emitted inside the ob loop.

            # ---- compute helpers ----
            adp_backlog = []   # (mt, ob, out_t) awaiting adapter matmul + add

            def base_mm(pb, mt, wt_t, t, start, stop):
                nc.tensor.matmul(
                    pb[:],
                    xh_sb[:, t, :, mt * P:(mt + 1) * P],
                    wt_t[:, t, :, :],
                    start=start, stop=stop,
                    perf_mode=DR,
                )

            def adapter_mm(ps, mt, ob, start, stop):
                nc.tensor.matmul(
                    ps[:],
                    sx_sb[:, :, mt * P:(mt + 1) * P],
                    badp_sb[:, ob, :, :],
                    start=start, stop=stop,
                    perf_mode=DR,
                )

            # px[h] = [32, 512]: rows 0-15 = x @ A_hi.T (hi+lo of x on lanes
            # 0-15), rows 16-31 = x_hi @ A_lo.T. Set A (x_hi against
            # [A_hi | A_lo]) is interleaved into ob0's waves below (it only
            # needs xh); set B (x_lo against A_hi) runs after ob2 when xl
            # has arrived.
            px_tiles = [
                psx.tile([2 * RANK, O_BLK], F32, tag="px", name="px0"),
                psx.tile([2 * RANK, O_BLK], F32, tag="px", name="px1"),
            ]

            def stage1_setA(t, start):
                for h in range(2):
                    nc.tensor.matmul(
                        px_tiles[h][:], at_sb[:, t, :, :],
                        xh_sb[:, t, :, h * O_BLK:(h + 1) * O_BLK],
                        start=start, stop=False, perf_mode=DR)

            def stage1_setB():
                for h in range(2):
                    px = px_tiles[h]
                    msl = slice(h * O_BLK, (h + 1) * O_BLK)
                    for t in range(KT2):
                        nc.tensor.matmul(
                            px[0:RANK, :], at_sb[:, t, :, 0:RANK],
                            xl_sb[:, t, :, msl],
                            start=False, stop=(t == KT2 - 1), perf_mode=DR)
                    # split 0.25*px into fp8 hi+lo, duplicated on rows 32-63
                    nc.vector.tensor_scalar_mul(tmp_sb[:], px[:], 0.25)
                    hi = sx_sb[0:2 * RANK, 0, msl]
                    lo = sx_sb[0:2 * RANK, 1, msl]
                    nc.vector.tensor_copy(out=hi, in_=tmp_sb[:])
                    nc.vector.tensor_tensor(out=lo, in0=tmp_sb[:], in1=hi,
                                            op=mybir.AluOpType.subtract)
                    nc.vector.tensor_copy(
                        out=sx_sb[2 * RANK:4 * RANK, 0, msl], in_=hi)
                    nc.vector.tensor_copy(
                        out=sx_sb[2 * RANK:4 * RANK, 1, msl], in_=lo)

            def store(out_t, mt, ob, eng=None):
                (eng or nc.sync).dma_start(
                    out_d[mt * P:(mt + 1) * P, ob * O_BLK:(ob + 1) * O_BLK],
                    out_t[:])

            def drain_backlog(k=1):
                for _ in range(k):
                    if not adp_backlog:
                        return
                    mt, ob, out_t = adp_backlog.pop(0)
                    pa = psa.tile([P, O_BLK], F32, tag="pa",
                                  name=f"pa{mt}_{ob}")
                    adapter_mm(pa, mt, ob, True, True)
                    nc.vector.tensor_tensor(
                        out=out_t[:], in0=out_t[:], in1=pa[:],
                        op=mybir.AluOpType.add)
                    store(out_t, mt, ob)

            # ---- PE program ----
            # obs 0-1: k-outer emission over mt groups of 5 (pool depth) so
            # matmuls start as soon as the wt/xh chunks land (DMA-paced
            # phase; keeps the PE as continuously busy as possible).
            for ob in range(2):
                wt_t = wt_tiles[ob]
                for mtq in (range(0, 6), range(6, 8)):
                    pbs = {}
                    for mt in mtq:
                        # borrow psa's bank (idle until ob3) for a 6-wide
                        # first wave so the PE keeps up with the DMA feed
                        pool = psa if mt == mtq.start + 5 else psb
                        pbs[mt] = pool.tile([P, O_BLK], F32,
                                            tag="pa" if pool is psa else "pb",
                                            name=f"pb{ob}_{mt}")
                    for t in range(KT2):
                        for mt in mtq:
                            base_mm(pbs[mt], mt, wt_t, t,
                                    start=(t == 0), stop=(t == KT2 - 1))
                        if ob == 0 and mtq.start == 0:
                            stage1_setA(t, start=(t == 0))
                    for mt in mtq:
                        out_t = outp.tile([P, O_BLK], BF16, tag="out",
                                          name=f"out{ob}_{mt}")
                        nc.scalar.copy(out=out_t[:], in_=pbs[mt][:])
                        adp_backlog.append((mt, ob, out_t))

            # obs 2-3: normal order, decoupled from the adapter.
            for ob in range(2, 4):
                wt_t = wt_tiles[ob]
                for mt in range(MT):
                    pb = psb.tile([P, O_BLK], F32, tag="pb",
                                  name=f"pb{ob}_{mt}")
                    for t in range(KT2):
                        base_mm(pb, mt, wt_t, t,
                                start=(t == 0), stop=(t == KT2 - 1))
                    out_t = outp.tile([P, O_BLK], BF16, tag="out",
                                      name=f"out{ob}_{mt}")
                    nc.scalar.copy(out=out_t[:], in_=pb[:])
                    adp_backlog.append((mt, ob, out_t))
                    if ob == 3 and mt >= 1:
                        drain_backlog(1)   # Sx is ready a few us into ob3
                if ob == 2:
                    stage1_setB()   # xl has arrived by now
                    load_wt(4)  # before ob3's stores hit the sync queue

            # obs 4-7: adapter accumulated in-group (17th matmul), backlog
            # drains interleaved.
            for ob in range(4, N_OBLK):
                if ob + 1 < N_OBLK and (ob + 1) not in wt_tiles:
                    load_wt(ob + 1)
                wt_t = wt_tiles[ob]
                for mt in range(MT):
                    pb = psb.tile([P, O_BLK], F32, tag="pb",
                                  name=f"pb{ob}_{mt}")
                    for t in range(KT2):
                        base_mm(pb, mt, wt_t, t,
                                start=(t == 0), stop=False)
                    adapter_mm(pb, mt, ob, False, True)
                    out_t = outp.tile([P, O_BLK], BF16, tag="out",
                                      name=f"out{ob}_{mt}")
                    nc.scalar.copy(out=out_t[:], in_=pb[:])
                    store(out_t, mt, ob)
                    drain_backlog(1)
            drain_backlog(len(adp_backlog))

    nc.compile()
    return nc


_NC_CACHE = None


def kernel(x, W, b, lora_A, lora_B, _trace=False):
    global LAST_RESULTS, _NC_CACHE

    scaling = ALPHA / RANK
    x = np.asarray(x, dtype=np.float32)
    W = np.asarray(W, dtype=np.float32)
    b = np.asarray(b, dtype=np.float32)
    A = np.asarray(lora_A, dtype=np.float32)
    B = np.asarray(lora_B, dtype=np.float32)

    # ---- weights (shared across cores) ----
    W8T = np.ascontiguousarray((W_SCALE * W).T).astype(NP_FP8)   # [IN_F, OUT_F]
    # wt[ob, p, t, i, o] = W8T[t*256 + i*128 + p, ob*512 + o]
    wt_in = np.ascontiguousarray(
        W8T.reshape(KT2, 2, P, N_OBLK, O_BLK).transpose(3, 2, 0, 1, 4)
    ).reshape(N_OBLK, P, KT2 * 2 * O_BLK)

    # at[p, t, i, 0:16] = A_hi[r, t*256+i*128+p]; [..., 16:32] = A_lo
    A8h = A.astype(NP_FP8)
    A8l = (A - A8h.astype(np.float32)).astype(NP_FP8)
    at_in = np.empty((P, KT2, 2, 2 * RANK), dtype=NP_FP8)
    ah = np.ascontiguousarray(
        A8h.T.reshape(KT2, 2, P, RANK).transpose(2, 0, 1, 3))
    al = np.ascontiguousarray(
        A8l.T.reshape(KT2, 2, P, RANK).transpose(2, 0, 1, 3))
    at_in[..., :RANK] = ah
    at_in[..., RANK:] = al
    at_in = at_in.reshape(P, KT2 * 2 * 2 * RANK)

    # badp[lane, ob, i, o]: lanes 0-15 & 16-31 pair with the two xa-parts
    # (both multiply B2_hi); 32-47 & 48-63 with B2_lo; 64 = bias (hi on i=0,
    # lo on i=1). Stage-2 stationary Sx carries 0.25*xa on lanes, so B ships
    # as 64*B (product = 16*xa*B = W_SCALE * scaling * xa @ B.T).
    B2 = (64.0 * B).astype(np.float32)             # [OUT_F, RANK]
    B2h = B2.astype(NP_FP8)
    B2l = (B2 - B2h.astype(np.float32)).astype(NP_FP8)
    b8 = (W_SCALE * b).astype(np.float32)
    b8h = b8.astype(NP_FP8)
    b8l = (b8 - b8h.astype(np.float32)).astype(NP_FP8)

    badp_in = np.zeros((SXP, N_OBLK, 2, O_BLK), dtype=NP_FP8)
    B2h_t = B2h.T.reshape(RANK, N_OBLK, O_BLK)     # [r, ob, o]
    B2l_t = B2l.T.reshape(RANK, N_OBLK, O_BLK)
    for i in range(2):
        badp_in[0:RANK, :, i, :] = B2h_t
        badp_in[RANK:2 * RANK, :, i, :] = B2h_t
        badp_in[2 * RANK:3 * RANK, :, i, :] = B2l_t
        badp_in[3 * RANK:4 * RANK, :, i, :] = B2l_t
    badp_in[SXP - 1, :, 0, :] = b8h.reshape(N_OBLK, O_BLK)
    badp_in[SXP - 1, :, 1, :] = b8l.reshape(N_OBLK, O_BLK)
    badp_in = badp_in.reshape(SXP, N_OBLK * 2 * O_BLK)

    ones_in = np.ones((1, 2 * M_PER_CORE), dtype=NP_FP8)

    # ---- per-core x shards (fp8 hi + lo) ----
    x_flat = np.ascontiguousarray(x.reshape(TOK, IN_F))

    def pack_x(a):
        # a: [IN_F, M] -> [p, t, i, m] flattened
        return np.ascontiguousarray(
            a.reshape(KT2, 2, P, M_PER_CORE).transpose(2, 0, 1, 3)
        ).reshape(P, KT2 * 2 * M_PER_CORE)

    in_maps = []
    for c in range(N_CORES):
        xs = np.ascontiguousarray(
            x_flat[c * M_PER_CORE:(c + 1) * M_PER_CORE].T)   # [IN_F, M]
        x8h = xs.astype(NP_FP8)
        x8l = (xs - x8h.astype(np.float32)).astype(NP_FP8)
        in_maps.append({
            "xh": pack_x(x8h),
            "xl": pack_x(x8l),
            "wt": wt_in,
            "at": at_in,
            "badp": badp_in,
            "ones": ones_in,
        })

    if _NC_CACHE is None:
        _NC_CACHE = _build_nc()
    nc = _NC_CACHE

    res = run_bass_kernel_spmd(nc, in_maps, core_ids=list(range(N_CORES)),
                               trace=_trace)
    LAST_RESULTS = res

    out = np.concatenate(
        [r["out"].astype(np.float32) for r in res.results], axis=0)
    out *= 1.0 / W_SCALE
    return out.reshape(B_SZ, S_SZ, OUT_F)


# revision 45
# speedup vs baseline: 3.5797x; 1.0187x over previous
"""LoRA Linear (x @ W.T + b + (alpha/rank) * (x @ A.T) @ B.T) on 8 TRN2 cores.

Strategy (v3, fp8 DoubleRow everywhere):
  - Data-parallel over tokens: 8192 tokens -> 8 shards of 1024 rows.
  - Base matmul x @ W.T in fp8e4 perf_mode=DoubleRow: each matmul contracts
    K=256 (128 partitions x 2 lanes) into a [128, 512] PSUM tile at 0.5
    cycles/row -- 4x fewer PE-array cycles than the bf16 baseline. 1024
    matmuls/core.
  - The LoRA adapter is NOT folded into W (fp8 quantization of the folded
    weight is too coarse: the rank-16 adapter dominates the output). Instead:
      stage 1: xaT = A-parts @ x directly in transposed layout: stationary =
               packed fp8 [A_hi | A_lo] (32 rank lanes), moving = x_hi (and a
               second set: A_hi x x_lo), DoubleRow, out [32, 512] PSUM.
               64 matmuls, no transposes needed.
      split:   DVE scales px by 0.25 and splits into an fp8 hi+lo pair,
               packed into the stage-2 stationary Sx [65, 2, 1024]
               (2 xa-parts x {hi,lo} x 16 ranks, + a ones lane for bias).
      stage 2: per out-tile: ONE DoubleRow fp8 matmul with 65 partitions
               (129 used lanes: xa-parts x B_{hi,lo} + bias), accumulated
               into the base PSUM group (ob >= 4) or a separate PSUM tile +
               DVE add (ob 0-3, whose base groups finish before Sx is ready).
  - x ships as an fp8 hi+lo pair (x ~ x_hi + x_lo): x_hi feeds the base
    matmul, hi+lo feed stage 1. Same DMA bytes as bf16 x.
  - W ships as fp8(8*W) to avoid e4m3 subnormals (W ~ N(0, 1/64)); B ships
    as fp8 hi/lo of 64*B, bias as fp8 hi/lo of 8*b, A as fp8 hi/lo of A.
    The device output is 8x the result in bf16; the host multiplies by
    0.125 (exact power-of-2) after gathering.
  - Scheduling: obs 0-1 are emitted k-outer across 6-wide mt waves (the 6th
    PSUM bank is borrowed from the adapter pool, idle until ob3) so the PE
    keeps pace with the DMA feed; stage-1 set A rides inside ob0's waves.
    DMA chunk sizes taper at the end of each feed-critical load so the last
    chunk's semaphore latency does not gate the consuming wave.
"""

import numpy as np

from concourse import bacc
import concourse.mybir as mybir
import concourse.tile as tile
from concourse.bass_utils import run_bass_kernel_spmd

N_CORES = 8
IN_F = 4096
OUT_F = 4096
RANK = 16
ALPHA = 32.0
B_SZ = 4
S_SZ = 2048
TOK = B_SZ * S_SZ            # 8192
M_PER_CORE = TOK // N_CORES  # 1024

P = 128
KT2 = IN_F // 256            # 16 DoubleRow k-tiles (256 contraction each)
O_BLK = 512
N_OBLK = OUT_F // O_BLK      # 8
MT = M_PER_CORE // P         # 8
SXP = 4 * RANK + 1           # 65 stage-2 stationary partitions

FP8 = mybir.dt.float8e4
NP_FP8 = mybir.dt.np(FP8)
BF16 = mybir.dt.bfloat16
NP_BF16 = mybir.dt.np(BF16)
F32 = mybir.dt.float32
DR = mybir.MatmulPerfMode.DoubleRow

W_SCALE = 8.0                # device computes 8x the result; host undoes it

LAST_RESULTS = None          # test.py reads exec_time_ns from here

NCHUNK = 4                   # DMA chunks for non-critical wt tensors
TPC = KT2 // NCHUNK          # kt2 tiles per chunk
# Tapered chunk schedule for the feed-critical loads: mostly 2 kt2-tiles per
# DMA, with small trailing chunks so the last-chunk completion (plus its
# 900ns semaphore latency) does not gate the consuming wave's end.
FEED_SIZES = [2, 2, 2, 2, 2, 2, 1, 1, 1, 1]
XL_SIZES = (6, 4, 3, 1, 1, 1)


def _build_nc():
    nc = bacc.Bacc(None, target_bir_lowering=False)

    xh_d = nc.dram_tensor("xh", [P, KT2 * 2 * M_PER_CORE], FP8, kind="ExternalInput")
    xl_d = nc.dram_tensor("xl", [P, KT2 * 2 * M_PER_CORE], FP8, kind="ExternalInput")
    wt_d = nc.dram_tensor("wt", [N_OBLK, P, KT2 * 2 * O_BLK], FP8, kind="ExternalInput")
    at_d = nc.dram_tensor("at", [P, KT2 * 2 * 2 * RANK], FP8, kind="ExternalInput")
    badp_d = nc.dram_tensor("badp", [SXP, N_OBLK * 2 * O_BLK], FP8, kind="ExternalInput")
    ones_d = nc.dram_tensor("ones", [1, 2 * M_PER_CORE], FP8, kind="ExternalInput")
    out_d = nc.dram_tensor("out", [M_PER_CORE, OUT_F], BF16, kind="ExternalOutput")

    with tile.TileContext(nc) as tc:
        with (
            tc.tile_pool(name="sb", bufs=1) as sb,
            tc.tile_pool(name="wtp", bufs=2) as wtp,
            tc.tile_pool(name="outp", bufs=36) as outp,
            tc.tile_pool(name="psb", bufs=5, space="PSUM") as psb,
            tc.tile_pool(name="psa", bufs=1, space="PSUM") as psa,
            tc.tile_pool(name="psx", bufs=2, space="PSUM") as psx,
        ):
            # ---- persistent SBUF tensors ----
            xh_sb = sb.tile([P, KT2, 2, M_PER_CORE], FP8, tag="xh")
            xl_sb = sb.tile([P, KT2, 2, M_PER_CORE], FP8, tag="xl")
            # at[:, t, :, 0:16] = A_hi lanes, [..., 16:32] = A_lo lanes
            at_sb = sb.tile([P, KT2, 2, 2 * RANK], FP8, tag="at")
            badp_sb = sb.tile([SXP, N_OBLK, 2, O_BLK], FP8, tag="badp")
            sx_sb = sb.tile([SXP, 2, M_PER_CORE], FP8, tag="sx")
            tmp_sb = sb.tile([2 * RANK, O_BLK], F32, tag="tmp")

            wt_tiles = {}

            def load_wt(ob, interleave_with=None, tpc=TPC,
                        after_first_chunk=None, sizes=None):
                t = wtp.tile([P, KT2, 2, O_BLK], FP8, tag="wt")
                if sizes is None:
                    sizes = [tpc] * (KT2 // tpc)
                t0 = 0
                for c, sz in enumerate(sizes):
                    tsl = slice(t0, t0 + sz)
                    if interleave_with is not None:
                        dst, src = interleave_with
                        xsz = 2 * M_PER_CORE
                        nc.sync.dma_start(
                            dst[:, tsl].rearrange("p t i m -> p (t i m)"),
                            src[:, t0 * xsz:(t0 + sz) * xsz])
                    wsz = 2 * O_BLK
                    nc.sync.dma_start(
                        t[:, tsl].rearrange("p t i m -> p (t i m)"),
                        wt_d[ob, :, t0 * wsz:(t0 + sz) * wsz],
                    )
                    if c == 0 and after_first_chunk is not None:
                        after_first_chunk()
                    t0 += sz
                assert t0 == KT2
                wt_tiles[ob] = t

            # ---- DMA priority order (sync queue is FIFO) ----
            # first xh/wt0 chunk pair ahead of the (small) at load so the
            # first matmuls start as early as possible
            load_wt(0, interleave_with=(xh_sb, xh_d),
                    sizes=FEED_SIZES,
                    after_first_chunk=lambda: nc.sync.dma_start(
                        at_sb[:].rearrange("p t i r -> p (t i r)"), at_d[:]))
            load_wt(1, sizes=FEED_SIZES)
            load_wt(2)
            t0 = 0
            for sz in XL_SIZES:
                xsz = 2 * M_PER_CORE
                nc.sync.dma_start(
                    xl_sb[:, t0:t0 + sz].rearrange("p t i m -> p (t i m)"),
                    xl_d[:, t0 * xsz:(t0 + sz) * xsz])
                t0 += sz
            assert t0 == KT2
            nc.sync.dma_start(
                badp_sb[:].rearrange("p ob i o -> p (ob i o)"), badp_d[:])
            nc.sync.dma_start(
                sx_sb[SXP - 1:SXP, :, :].rearrange("p i m -> p (i m)"), ones_d[:])
            load_wt(3)
            # wt4..wt7 are emitted inside the ob loop.

            # ---- compute helpers ----
            adp_backlog = []   # (mt, ob, out_t) awaiting adapter matmul + add

            def base_mm(pb, mt, wt_t, t, start, stop):
                nc.tensor.matmul(
                    pb[:],
                    xh_sb[:, t, :, mt * P:(mt + 1) * P],
                    wt_t[:, t, :, :],
                    start=start, stop=stop,
                    perf_mode=DR,
                )

            def adapter_mm(ps, mt, ob, start, stop):
                nc.tensor.matmul(
                    ps[:],
                    sx_sb[:, :, mt * P:(mt + 1) * P],
                    badp_sb[:, ob, :, :],
                    start=start, stop=stop,
                    perf_mode=DR,
                )

            # px[h] = [32, 512]: rows 0-15 = x @ A_hi.T (hi+lo of x on lanes
            # 0-15), rows 16-31 = x_hi @ A_lo.T. Set A (x_hi against
            # [A_hi | A_lo]) is interleaved into ob0's waves below (it only
            # needs xh); set B (x_lo against A_hi) runs after ob2 when xl
            # has arrived.
            px_tiles = [
                psx.tile([2 * RANK, O_BLK], F32, tag="px", name="px0"),
                psx.tile([2 * RANK, O_BLK], F32, tag="px", name="px1"),
            ]

            def stage1_setA(t, start):
                for h in range(2):
                    nc.tensor.matmul(
                        px_tiles[h][:], at_sb[:, t, :, :],
                        xh_sb[:, t, :, h * O_BLK:(h + 1) * O_BLK],
                        start=start, stop=False, perf_mode=DR)

            def stage1_setB():
                for h in range(2):
                    px = px_tiles[h]
                    msl = slice(h * O_BLK, (h + 1) * O_BLK)
                    for t in range(KT2):
                        nc.tensor.matmul(
                            px[0:RANK, :], at_sb[:, t, :, 0:RANK],
                            xl_sb[:, t, :, msl],
                            start=False, stop=(t == KT2 - 1), perf_mode=DR)
                    # split 0.25*px into fp8 hi+lo, duplicated on rows 32-63
                    nc.vector.tensor_scalar_mul(tmp_sb[:], px[:], 0.25)
                    hi = sx_sb[0:2 * RANK, 0, msl]
                    lo = sx_sb[0:2 * RANK, 1, msl]
                    nc.vector.tensor_copy(out=hi, in_=tmp_sb[:])
                    nc.vector.tensor_tensor(out=lo, in0=tmp_sb[:], in1=hi,
                                            op=mybir.AluOpType.subtract)
                    nc.vector.tensor_copy(
                        out=sx_sb[2 * RANK:4 * RANK, 0, msl], in_=hi)
                    nc.vector.tensor_copy(
                        out=sx_sb[2 * RANK:4 * RANK, 1, msl], in_=lo)

            def store(out_t, mt, ob, eng=None):
                (eng or nc.sync).dma_start(
                    out_d[mt * P:(mt + 1) * P, ob * O_BLK:(ob + 1) * O_BLK],
                    out_t[:])

            def drain_backlog(k=1):
                for _ in range(k):
                    if not adp_backlog:
                        return
                    mt, ob, out_t = adp_backlog.pop(0)
                    pa = psa.tile([P, O_BLK], F32, tag="pa",
                                  name=f"pa{mt}_{ob}")
                    adapter_mm(pa, mt, ob, True, True)
                    nc.vector.tensor_tensor(
                        out=out_t[:], in0=out_t[:], in1=pa[:],
                        op=mybir.AluOpType.add)
                    store(out_t, mt, ob)

            # ---- PE program ----
            # obs 0-1: k-outer emission over 6-wide mt waves so matmuls start
            # as soon as the wt/xh chunks land (DMA-paced phase). The 6th
            # wave slot borrows psa's bank (idle until ob3).
            for ob in range(2):
                wt_t = wt_tiles[ob]
                for mtq in (range(0, 6), range(6, 8)):
                    pbs = {}
                    for mt in mtq:
                        pool = psa if mt == mtq.start + 5 else psb
                        pbs[mt] = pool.tile([P, O_BLK], F32,
                                            tag="pa" if pool is psa else "pb",
                                            name=f"pb{ob}_{mt}")
                    for t in range(KT2):
                        for mt in mtq:
                            base_mm(pbs[mt], mt, wt_t, t,
                                    start=(t == 0), stop=(t == KT2 - 1))
                        if ob == 0 and mtq.start == 0:
                            stage1_setA(t, start=(t == 0))
                    for mt in mtq:
                        out_t = outp.tile([P, O_BLK], BF16, tag="out",
                                          name=f"out{ob}_{mt}")
                        nc.scalar.copy(out=out_t[:], in_=pbs[mt][:])
                        adp_backlog.append((mt, ob, out_t))

            # obs 2-3: normal order, still decoupled from the adapter.
            for ob in range(2, 4):
                wt_t = wt_tiles[ob]
                for mt in range(MT):
                    pb = psb.tile([P, O_BLK], F32, tag="pb",
                                  name=f"pb{ob}_{mt}")
                    for t in range(KT2):
                        base_mm(pb, mt, wt_t, t,
                                start=(t == 0), stop=(t == KT2 - 1))
                    out_t = outp.tile([P, O_BLK], BF16, tag="out",
                                      name=f"out{ob}_{mt}")
                    nc.scalar.copy(out=out_t[:], in_=pb[:])
                    adp_backlog.append((mt, ob, out_t))
                    if ob == 3 and mt >= 1:
                        drain_backlog(1)   # Sx is ready a few us into ob3
                if ob == 2:
                    stage1_setB()   # xl has arrived by now
                    load_wt(4)  # before ob3's stores hit the sync queue

            # obs 4-7: adapter accumulated in-group (17th matmul), backlog
            # drains interleaved.
            for ob in range(4, N_OBLK):
                if ob + 1 < N_OBLK and (ob + 1) not in wt_tiles:
                    load_wt(ob + 1)
                wt_t = wt_tiles[ob]
                for mt in range(MT):
                    pb = psb.tile([P, O_BLK], F32, tag="pb",
                                  name=f"pb{ob}_{mt}")
                    for t in range(KT2):
                        base_mm(pb, mt, wt_t, t,
                                start=(t == 0), stop=False)
                    adapter_mm(pb, mt, ob, False, True)
                    out_t = outp.tile([P, O_BLK], BF16, tag="out",
                                      name=f"out{ob}_{mt}")
                    nc.scalar.copy(out=out_t[:], in_=pb[:])
                    store(out_t, mt, ob)
                    drain_backlog(1)
            drain_backlog(len(adp_backlog))

    nc.compile()
    return nc


_NC_CACHE = None


def kernel(x, W, b, lora_A, lora_B, _trace=False):
    global LAST_RESULTS, _NC_CACHE

    scaling = ALPHA / RANK
    x = np.asarray(x, dtype=np.float32)
    W = np.asarray(W, dtype=np.float32)
    b = np.asarray(b, dtype=np.float32)
    A = np.asarray(lora_A, dtype=np.float32)
    B = np.asarray(lora_B, dtype=np.float32)

    # ---- weights (shared across cores) ----
    W8T = np.ascontiguousarray((W_SCALE * W).T).astype(NP_FP8)   # [IN_F, OUT_F]
    # wt[ob, p, t, i, o] = W8T[t*256 + i*128 + p, ob*512 + o]
    wt_in = np.ascontiguousarray(
        W8T.reshape(KT2, 2, P, N_OBLK, O_BLK).transpose(3, 2, 0, 1, 4)
    ).reshape(N_OBLK, P, KT2 * 2 * O_BLK)

    # at[p, t, i, 0:16] = A_hi[r, t*256+i*128+p]; [..., 16:32] = A_lo
    A8h = A.astype(NP_FP8)
    A8l = (A - A8h.astype(np.float32)).astype(NP_FP8)
    at_in = np.empty((P, KT2, 2, 2 * RANK), dtype=NP_FP8)
    at_in[..., :RANK] = np.ascontiguousarray(
        A8h.T.reshape(KT2, 2, P, RANK).transpose(2, 0, 1, 3))
    at_in[..., RANK:] = np.ascontiguousarray(
        A8l.T.reshape(KT2, 2, P, RANK).transpose(2, 0, 1, 3))
    at_in = at_in.reshape(P, KT2 * 2 * 2 * RANK)

    # badp[lane, ob, i, o]: lanes 0-15 & 16-31 pair with the two xa-parts
    # (both multiply B2_hi); 32-47 & 48-63 with B2_lo; 64 = bias (hi on i=0,
    # lo on i=1). Stage-2 stationary Sx carries 0.25*xa on its lanes, so B
    # ships as 64*B (product = 16*xa*B = W_SCALE * scaling * xa @ B.T).
    B2 = (64.0 * B).astype(np.float32)             # [OUT_F, RANK]
    B2h = B2.astype(NP_FP8)
    B2l = (B2 - B2h.astype(np.float32)).astype(NP_FP8)
    b8 = (W_SCALE * b).astype(np.float32)
    b8h = b8.astype(NP_FP8)
    b8l = (b8 - b8h.astype(np.float32)).astype(NP_FP8)

    badp_in = np.zeros((SXP, N_OBLK, 2, O_BLK), dtype=NP_FP8)
    B2h_t = B2h.T.reshape(RANK, N_OBLK, O_BLK)     # [r, ob, o]
    B2l_t = B2l.T.reshape(RANK, N_OBLK, O_BLK)
    for i in range(2):
        badp_in[0:RANK, :, i, :] = B2h_t
        badp_in[RANK:2 * RANK, :, i, :] = B2h_t
        badp_in[2 * RANK:3 * RANK, :, i, :] = B2l_t
        badp_in[3 * RANK:4 * RANK, :, i, :] = B2l_t
    badp_in[SXP - 1, :, 0, :] = b8h.reshape(N_OBLK, O_BLK)
    badp_in[SXP - 1, :, 1, :] = b8l.reshape(N_OBLK, O_BLK)
    badp_in = badp_in.reshape(SXP, N_OBLK * 2 * O_BLK)

    ones_in = np.ones((1, 2 * M_PER_CORE), dtype=NP_FP8)

    # ---- per-core x shards (fp8 hi + lo) ----
    x_flat = np.ascontiguousarray(x.reshape(TOK, IN_F))

    def pack_x(a):
        # a: [IN_F, M] -> [p, t, i, m] flattened
        return np.ascontiguousarray(
            a.reshape(KT2, 2, P, M_PER_CORE).transpose(2, 0, 1, 3)
        ).reshape(P, KT2 * 2 * M_PER_CORE)

    in_maps = []
    for c in range(N_CORES):
        xs = np.ascontiguousarray(
            x_flat[c * M_PER_CORE:(c + 1) * M_PER_CORE].T)   # [IN_F, M]
        x8h = xs.astype(NP_FP8)
        x8l = (xs - x8h.astype(np.float32)).astype(NP_FP8)
        in_maps.append({
            "xh": pack_x(x8h),
            "xl": pack_x(x8l),
            "wt": wt_in,
            "at": at_in,
            "badp": badp_in,
            "ones": ones_in,
        })

    if _NC_CACHE is None:
        _NC_CACHE = _build_nc()
    nc = _NC_CACHE

    res = run_bass_kernel_spmd(nc, in_maps, core_ids=list(range(N_CORES)),
                               trace=_trace)
    LAST_RESULTS = res

    out = np.concatenate(
        [r["out"].astype(np.float32) for r in res.results], axis=0)
    out *= 1.0 / W_SCALE
    return out.reshape(B_SZ, S_SZ, OUT_F)
